# revision 10
# baseline (speedup 1.0000x reference)
"""Trainium2 Bass kernel for the CGF tree-GRU problem.

Problem: 3-level complete 8-ary tree GRU (torch GRU cell convention).
  Level 3: 64 nodes x 8 embedded leaf children, h0 = 0
  Level 2:  8 nodes x 8 children (level-3 outputs), h0 = mean of children h
  Level 1:  1 node  x 8 children (level-2 outputs), h0 = mean of children h
  Output: mean over the 8 step outputs of the root GRU. D = 512.

Distribution choice: the computation is ONE serial chain of 24 GRU steps
(8 per level; levels strictly dependent).  Each step is dominated by moving
W_hh (1536x512) through the PE array, independent of the node-batch size, so
sharding the node batch across cores saves nothing, and sharding the hidden
dim requires a per-step all-gather whose latency exceeds a whole step.  The
kernel is therefore replicated on all 8 cores (SPMD, identical inputs); core
0's output is returned.

Layout: everything lives TRANSPOSED on chip - gate/hidden dims on the 128
partitions (4 or 12 tiles of 128), batch on the free dim.  This makes GRU
biases per-partition scalars (fused into activation/scalar_tensor_tensor
ops), halves DVE cost vs. the natural layout, and removes all transposes:
the recurrent matmul gh^T = W_hh @ h^T consumes h^T directly, and each
level's mean-output feeds the next level's input matmul without reshaping.
"""

import numpy as np

import concourse.bacc as bacc
import concourse.bass as bass
import concourse.mybir as mybir
from concourse.tile import TileContext
from concourse.bass_utils import run_bass_kernel_spmd

AF = mybir.ActivationFunctionType
OP = mybir.AluOpType
FP = mybir.dt.float32

P = 128          # partitions
D = 512          # hidden size
KT = D // P      # 4 k-tiles (contraction)
G = 3 * D        # 1536 gate dims
MT = G // P      # 12 m-tiles (gate rows)
A = 8            # tree arity == sequence length per level
NB = 64          # level-3 node count
T = 8            # steps per level
N_CORES = 8
BLOB_COLS = 2 * MT * KT * P + KT * T * NB + MT + KT + KT * NB

_BUILT = None  # cached Bass module


def _v(ap, g):
    """View a 2-D [P, g*b] AP as [P, g, b]."""
    return ap.rearrange("p (g b) -> p g b", g=g)


def _build_nc():
    nc = bacc.Bacc()

    blob = nc.declare_dram_parameter("blob", [P, BLOB_COLS], FP, isOutput=False)
    outp = nc.declare_dram_parameter("out", [P, KT], FP, isOutput=True)

    with TileContext(nc) as tc:
        with (
            tc.tile_pool(name="const", bufs=1) as cpool,
            tc.tile_pool(name="state", bufs=1) as spool,
            tc.tile_pool(name="work", bufs=2) as wpool,
            tc.tile_pool(name="pg", bufs=2, space="PSUM") as gpool,
            tc.tile_pool(name="prz", bufs=2, space="PSUM") as rzpool,
            tc.tile_pool(name="pn", bufs=2, space="PSUM") as npool,
        ):
            # Inputs arrive as one packed blob, DMA'd in 512-col chunks:
            # a wide DMA fans out over many HW-DGE queues and its consumers
            # then exceed the per-instruction sync-wait slot budget, while a
            # 2KB-per-partition chunk stays narrow.  Chunk boundaries align
            # with every consumer slice (all are 512-col aligned), and tile
            # dependency tracking is range-based, so each consumer waits on
            # exactly the chunks it reads.
            blob_sb = cpool.tile([P, BLOB_COLS], FP)
            for c0 in range(0, BLOB_COLS, 512):
                c1 = min(c0 + 512, BLOB_COLS)
                nc.sync.dma_start(out=blob_sb[:, c0:c1], in_=blob[:, c0:c1])
            o = 0
            wit_sb = blob_sb[:, o : o + MT * KT * P]; o += MT * KT * P
            wht_sb = blob_sb[:, o : o + MT * KT * P]; o += MT * KT * P
            xt_sb = blob_sb[:, o : o + KT * T * NB]; o += KT * T * NB
            gb_sb = blob_sb[:, o : o + MT]; o += MT
            bhn_sb = blob_sb[:, o : o + KT]; o += KT
            bhnb_sb = blob_sb[:, o : o + KT * NB]; o += KT * NB
            assert o == BLOB_COLS

            def compute_gi(gi_tile, rhs_of_k, ncols):
                """gi^T = W_ih @ x^T + combined bias, m-tile at a time.

                gi_tile: [P, MT*ncols]; rhs_of_k(k) -> [P, ncols] AP of x^T.
                """
                for m in range(MT):
                    ps = gpool.tile([P, ncols], FP, tag="gi_ps")
                    for k in range(KT):
                        nc.tensor.matmul(
                            ps[:, :],
                            lhsT=wit_sb[:, (m * KT + k) * P : (m * KT + k + 1) * P],
                            rhs=rhs_of_k(k),
                            start=(k == 0),
                            stop=(k == KT - 1),
                        )
                    # PSUM -> SBUF copy with the per-gate-row bias folded in.
                    nc.scalar.activation(
                        gi_tile[:, m * ncols : (m + 1) * ncols],
                        ps[:, :],
                        AF.Identity,
                        bias=gb_sb[:, m : m + 1],
                        scale=1.0,
                    )

            def gru_level(B, h_tile, acc_tile, gi_rz_at, gi_n_at, zero_h0):
                """Run 8 GRU steps.  h_tile [P, KT*B] is h^T (written in
                place each step; must hold h0 unless zero_h0), acc_tile
                accumulates the step outputs.  gi_*_at(t) -> [P, g, B] APs.
                """
                for t in range(T):
                    if t == 0 and zero_h0:
                        # h = 0 so gh == b_hh exactly; skip the matmuls.
                        rzt = wpool.tile([P, 8 * B], FP, tag="rz")
                        nc.scalar.activation(_v(rzt[:], 8), gi_rz_at(t), AF.Sigmoid)
                        bt = wpool.tile([P, KT * B], FP, tag="bt")
                        nc.vector.tensor_mul(
                            _v(bt[:], KT),
                            _v(rzt[:, : KT * B], KT),
                            _v(bhnb_sb, KT)[:, :, :B],
                        )
                        ct = wpool.tile([P, KT * B], FP, tag="ct")
                        nc.vector.tensor_add(_v(ct[:], KT), _v(bt[:], KT), gi_n_at(t))
                        nt = wpool.tile([P, KT * B], FP, tag="nt")
                        nc.scalar.activation(nt[:, :], ct[:, :], AF.Tanh)
                        # h1 = (1 - z) * n = n - z*n
                        ft = wpool.tile([P, KT * B], FP, tag="ft")
                        nc.vector.tensor_mul(ft[:, :], rzt[:, KT * B :], nt[:, :])
                        nc.vector.tensor_sub(h_tile[:, :], nt[:, :], ft[:, :])
                        nc.vector.tensor_copy(acc_tile[:, :], h_tile[:, :])
                        continue

                    ps_rz = rzpool.tile([P, 8 * B], FP, tag="ps_rz")
                    ps_n = npool.tile([P, KT * B], FP, tag="ps_n")
                    for m in range(MT):
                        if m < 8:
                            dst = ps_rz[:, m * B : (m + 1) * B]
                        else:
                            dst = ps_n[:, (m - 8) * B : (m - 7) * B]
                        for k in range(KT):
                            nc.tensor.matmul(
                                dst,
                                lhsT=wht_sb[:, (m * KT + k) * P : (m * KT + k + 1) * P],
                                rhs=h_tile[:, k * B : (k + 1) * B],
                                start=(k == 0),
                                stop=(k == KT - 1),
                            )
                    arz = wpool.tile([P, 8 * B], FP, tag="arz")
                    nc.vector.tensor_add(_v(arz[:], 8), _v(ps_rz[:], 8), gi_rz_at(t))
                    rzt = wpool.tile([P, 8 * B], FP, tag="rz")
                    nc.scalar.activation(rzt[:, :], arz[:, :], AF.Sigmoid)
                    # b = (gh_n + b_hn) * r, per k-chunk (bias differs per chunk)
                    bt = wpool.tile([P, KT * B], FP, tag="bt")
                    for k in range(KT):
                        nc.vector.scalar_tensor_tensor(
                            bt[:, k * B : (k + 1) * B],
                            in0=ps_n[:, k * B : (k + 1) * B],
                            scalar=bhn_sb[:, k : k + 1],
                            in1=rzt[:, k * B : (k + 1) * B],
                            op0=OP.add,
                            op1=OP.mult,
                        )
                    ct = wpool.tile([P, KT * B], FP, tag="ct")
                    nc.vector.tensor_add(_v(ct[:], KT), _v(bt[:], KT), gi_n_at(t))
                    nt = wpool.tile([P, KT * B], FP, tag="nt")
                    nc.scalar.activation(nt[:, :], ct[:, :], AF.Tanh)
                    # h' = n + z*(h - n)
                    et = wpool.tile([P, KT * B], FP, tag="et")
                    nc.vector.tensor_sub(et[:, :], h_tile[:, :], nt[:, :])
                    ft = wpool.tile([P, KT * B], FP, tag="ft")
                    nc.vector.tensor_mul(ft[:, :], rzt[:, KT * B :], et[:, :])
                    nc.vector.tensor_add(h_tile[:, :], nt[:, :], ft[:, :])
                    if t == 0:
                        nc.vector.tensor_copy(acc_tile[:, :], h_tile[:, :])
                    else:
                        nc.vector.tensor_add(acc_tile[:, :], acc_tile[:, :], h_tile[:, :])

            # ---------------- Level 3: 64 nodes ----------------
            gi3 = cpool.tile([P, MT * T * NB], FP)
            # moving operand is 512 cols - exactly the fp32 cap
            compute_gi(gi3, lambda k: xt_sb[:, k * (T * NB) : (k + 1) * (T * NB)], T * NB)
            gi3v = gi3[:].rearrange("p (m t b) -> p m t b", m=MT, t=T)
            h3 = spool.tile([P, KT * NB], FP)
            acc3 = spool.tile([P, KT * NB], FP)
            gru_level(
                NB,
                h3,
                acc3,
                lambda t: gi3v[:, 0:8, t],
                lambda t: gi3v[:, 8:12, t],
                zero_h0=True,
            )

            # ---------------- Level 3 -> 2 transition ----------------
            x2 = spool.tile([P, KT * NB], FP)
            nc.scalar.mul(x2[:, :], acc3[:, :], 1.0 / A)
            h2 = spool.tile([P, KT * A], FP)
            hr2 = spool.tile([P, KT * A], FP)
            nc.vector.tensor_reduce(
                _v(hr2[:], KT),
                h3[:].rearrange("p (k j c) -> p k j c", k=KT, j=A),
                axis=mybir.AxisListType.X,
                op=OP.add,
            )
            nc.scalar.mul(h2[:, :], hr2[:, :], 1.0 / A)

            gi2 = cpool.tile([P, MT * NB], FP)
            compute_gi(gi2, lambda k: x2[:, k * NB : (k + 1) * NB], NB)
            # gi2 within-m column order is (j, t) - natural child order; the
            # step-t slice is strided.
            gi2v = gi2[:].rearrange("p (m j t) -> p m j t", m=MT, j=A)
            acc2 = spool.tile([P, KT * A], FP)
            gru_level(
                A,
                h2,
                acc2,
                lambda t: gi2v[:, 0:8, :, t],
                lambda t: gi2v[:, 8:12, :, t],
                zero_h0=False,
            )

            # ---------------- Level 2 -> 1 transition ----------------
            x1 = spool.tile([P, KT * A], FP)
            nc.scalar.mul(x1[:, :], acc2[:, :], 1.0 / A)
            h1 = spool.tile([P, KT], FP)
            hr1 = spool.tile([P, KT], FP)
            nc.vector.tensor_reduce(
                _v(hr1[:], KT),
                h2[:].rearrange("p (k j c) -> p k j c", k=KT, j=1),
                axis=mybir.AxisListType.X,
                op=OP.add,
            )
            nc.scalar.mul(h1[:, :], hr1[:, :], 1.0 / A)

            gi1 = cpool.tile([P, MT * A], FP)
            compute_gi(gi1, lambda k: x1[:, k * A : (k + 1) * A], A)
            gi1v = gi1[:].rearrange("p (m t) -> p m t", m=MT)
            acc1 = spool.tile([P, KT], FP)
            gru_level(
                1,
                h1,
                acc1,
                lambda t: gi1v[:, 0:8, t : t + 1],
                lambda t: gi1v[:, 8:12, t : t + 1],
                zero_h0=False,
            )

            out_sb = spool.tile([P, KT], FP)
            nc.scalar.mul(out_sb[:, :], acc1[:, :], 1.0 / A)
            nc.sync.dma_start(out=outp[:, :], in_=out_sb[:, :])

    nc.finalize()
    return nc


def _get_nc():
    global _BUILT
    if _BUILT is None:
        _BUILT = _build_nc()
    return _BUILT


def make_inputs(leaf_ids, embed_table, W_ih, W_hh, b_ih, b_hh):
    """Host-side shard/layout prep: gather the looked-up embedding rows and
    lay every tensor out in the on-chip transposed format."""
    leaf_ids = np.asarray(leaf_ids).astype(np.int64)
    emb = np.asarray(embed_table, dtype=np.float32)
    W_ih = np.asarray(W_ih, dtype=np.float32)
    W_hh = np.asarray(W_hh, dtype=np.float32)
    b_ih = np.asarray(b_ih, dtype=np.float32)
    b_hh = np.asarray(b_hh, dtype=np.float32)

    x = emb[leaf_ids]  # [64, 8, 512]
    # time-major batch: row b = t*64 + node
    xtm = np.ascontiguousarray(x.transpose(1, 0, 2)).reshape(T * NB, D)
    xt_in = np.ascontiguousarray(
        xtm.T.reshape(KT, P, T * NB).transpose(1, 0, 2)
    ).reshape(P, KT * T * NB)

    def pack_w(W):  # W [1536, 512] -> lhsT tiles [(m,k) major]
        WT = np.ascontiguousarray(W.T)  # [512, 1536]
        return np.ascontiguousarray(
            WT.reshape(KT, P, MT, P).transpose(1, 2, 0, 3)
        ).reshape(P, MT * KT * P)

    gbias = np.concatenate([(b_ih + b_hh)[: 2 * D], b_ih[2 * D :]])
    gb_in = np.ascontiguousarray(gbias.reshape(MT, P).T)
    bhn_in = np.ascontiguousarray(b_hh[2 * D :].reshape(KT, P).T)
    bhnb_in = np.ascontiguousarray(np.repeat(bhn_in, NB, axis=1))

    blob = np.concatenate(
        [pack_w(W_ih), pack_w(W_hh), xt_in, gb_in, bhn_in, bhnb_in], axis=1
    )
    assert blob.shape == (P, BLOB_COLS)
    return {"blob": np.ascontiguousarray(blob)}


def unpack_output(out_np):
    # out [P, KT]: element (p, k) = root dim k*128+p
    return np.ascontiguousarray(out_np.T).reshape(1, 1, D).astype(np.float32)


def kernel(leaf_ids=None, layer=None, embed_table=None, W_ih=None, W_hh=None,
           b_ih=None, b_hh=None, **_unused):
    in_map = make_inputs(leaf_ids, embed_table, W_ih, W_hh, b_ih, b_hh)
    nc = _get_nc()
    res = run_bass_kernel_spmd(nc, [in_map] * N_CORES, list(range(N_CORES)))
    return unpack_output(res.results[0]["out"])


# revision 12
# speedup vs baseline: 3.9495x; 3.9495x over previous
"""Trainium2 Bass kernel for the CGF tree-GRU problem.

Problem: 3-level complete 8-ary tree GRU (torch GRU cell convention).
  Level 3: 64 nodes x 8 embedded leaf children, h0 = 0
  Level 2:  8 nodes x 8 children (level-3 outputs), h0 = mean of children h
  Level 1:  1 node  x 8 children (level-2 outputs), h0 = mean of children h
  Output: mean over the 8 step outputs of the root GRU. D = 512.

Distribution choice: the computation is ONE serial chain of 24 GRU steps
(8 per level; levels strictly dependent).  Each step is dominated by moving
W_hh (1536x512) through the PE array, independent of the node-batch size, so
sharding the node batch across cores saves nothing, and sharding the hidden
dim requires a per-step all-gather whose latency exceeds a whole step.  The
kernel is therefore replicated on all 8 cores (SPMD, identical inputs); core
0's output is returned.

Layout: everything lives TRANSPOSED on chip - gate/hidden dims on the 128
partitions (4 or 12 tiles of 128), batch on the free dim.  This makes GRU
biases per-partition scalars (fused into activation/scalar_tensor_tensor
ops), halves DVE cost vs. the natural layout, and removes all transposes:
the recurrent matmul gh^T = W_hh @ h^T consumes h^T directly, and each
level's mean-output feeds the next level's input matmul without reshaping.
"""

import numpy as np

import concourse.bacc as bacc
import concourse.bass as bass
import concourse.mybir as mybir
from concourse.tile import TileContext
from concourse.bass_utils import run_bass_kernel_spmd

AF = mybir.ActivationFunctionType
OP = mybir.AluOpType
FP = mybir.dt.float32
BF = mybir.dt.bfloat16

P = 128          # partitions
D = 512          # hidden size
KT = D // P      # 4 k-tiles (contraction)
G = 3 * D        # 1536 gate dims
MT = G // P      # 12 m-tiles (gate rows)
A = 8            # tree arity == sequence length per level
NB = 64          # level-3 node count
T = 8            # steps per level
N_CORES = 8
B16_COLS = 2 * MT * KT * P + KT * T * NB   # bf16 blob: wit, wht, xt
B32_COLS = MT + KT + KT * NB               # fp32 blob: gb, bhn, bhnb

_BUILT = None  # cached Bass module


def _v(ap, g):
    """View a 2-D [P, g*b] AP as [P, g, b]."""
    return ap.rearrange("p (g b) -> p g b", g=g)


def _build_nc():
    nc = bacc.Bacc()

    blob16 = nc.declare_dram_parameter("blob16", [P, B16_COLS], BF, isOutput=False)
    blob32 = nc.declare_dram_parameter("blob32", [P, B32_COLS], FP, isOutput=False)
    outp = nc.declare_dram_parameter("out", [P, KT], FP, isOutput=True)

    with TileContext(nc) as tc:
        with (
            tc.tile_pool(name="const", bufs=1) as cpool,
            tc.tile_pool(name="state", bufs=1) as spool,
            tc.tile_pool(name="work", bufs=2) as wpool,
            tc.tile_pool(name="pg", bufs=2, space="PSUM") as gpool,
            tc.tile_pool(name="prz", bufs=2, space="PSUM") as rzpool,
            tc.tile_pool(name="pn", bufs=2, space="PSUM") as npool,
        ):
            # Inputs arrive as one packed blob, DMA'd in 512-col chunks:
            # a wide DMA fans out over many HW-DGE queues and its consumers
            # then exceed the per-instruction sync-wait slot budget, while a
            # 2KB-per-partition chunk stays narrow.  Chunk boundaries align
            # with every consumer slice (all are 512-col aligned), and tile
            # dependency tracking is range-based, so each consumer waits on
            # exactly the chunks it reads.
            b16_sb = cpool.tile([P, B16_COLS], BF)
            for c0 in range(0, B16_COLS, 1024):
                c1 = min(c0 + 1024, B16_COLS)
                nc.sync.dma_start(out=b16_sb[:, c0:c1], in_=blob16[:, c0:c1])
            b32_sb = cpool.tile([P, B32_COLS], FP)
            nc.sync.dma_start(out=b32_sb[:], in_=blob32[:, :])
            o = 0
            wit_sb = b16_sb[:, o : o + MT * KT * P]; o += MT * KT * P
            wht_sb = b16_sb[:, o : o + MT * KT * P]; o += MT * KT * P
            xt_sb = b16_sb[:, o : o + KT * T * NB]; o += KT * T * NB
            assert o == B16_COLS
            o = 0
            gb_sb = b32_sb[:, o : o + MT]; o += MT
            bhn_sb = b32_sb[:, o : o + KT]; o += KT
            bhnb_sb = b32_sb[:, o : o + KT * NB]; o += KT * NB
            assert o == B32_COLS

            def compute_gi(gi_tile, rhs_of_k, ncols):
                """gi^T = W_ih @ x^T + combined bias, m-tile at a time.

                gi_tile: [P, MT*ncols]; rhs_of_k(k) -> [P, ncols] AP of x^T.
                """
                for m in range(MT):
                    ps = gpool.tile([P, ncols], FP, tag="gi_ps")
                    for k in range(KT):
                        nc.tensor.matmul(
                            ps[:, :],
                            lhsT=wit_sb[:, (m * KT + k) * P : (m * KT + k + 1) * P],
                            rhs=rhs_of_k(k),
                            start=(k == 0),
                            stop=(k == KT - 1),
                        )
                    # PSUM -> SBUF copy with the per-gate-row bias folded in.
                    nc.scalar.activation(
                        gi_tile[:, m * ncols : (m + 1) * ncols],
                        ps[:, :],
                        AF.Identity,
                        bias=gb_sb[:, m : m + 1],
                        scale=1.0,
                    )

            def gru_level(B, h_tile, acc_tile, gi_rz_at, gi_n_at, zero_h0):
                """Run 8 GRU steps.  h_tile [P, KT*B] is h^T (written in
                place each step; must hold h0 unless zero_h0), acc_tile
                accumulates the step outputs.  gi_*_at(t) -> [P, g, B] APs.
                """
                for t in range(T):
                    if t == 0 and zero_h0:
                        # h = 0 so gh == b_hh exactly; skip the matmuls.
                        rzt = wpool.tile([P, 8 * B], FP, tag="rz")
                        nc.scalar.activation(_v(rzt[:], 8), gi_rz_at(t), AF.Sigmoid)
                        bt = wpool.tile([P, KT * B], FP, tag="bt")
                        nc.vector.tensor_mul(
                            _v(bt[:], KT),
                            _v(rzt[:, : KT * B], KT),
                            _v(bhnb_sb, KT)[:, :, :B],
                        )
                        ct = wpool.tile([P, KT * B], FP, tag="ct")
                        nc.vector.tensor_add(_v(ct[:], KT), _v(bt[:], KT), gi_n_at(t))
                        nt = wpool.tile([P, KT * B], FP, tag="nt")
                        nc.scalar.activation(nt[:, :], ct[:, :], AF.Tanh)
                        # h1 = (1 - z) * n = n - z*n
                        ft = wpool.tile([P, KT * B], FP, tag="ft")
                        nc.vector.tensor_mul(ft[:, :], rzt[:, KT * B :], nt[:, :])
                        nc.vector.tensor_sub(h_tile[:, :], nt[:, :], ft[:, :])
                        nc.vector.tensor_copy(acc_tile[:, :], h_tile[:, :])
                        continue

                    ps_rz = rzpool.tile([P, 8 * B], FP, tag="ps_rz")
                    ps_n = npool.tile([P, KT * B], FP, tag="ps_n")
                    for m in range(MT):
                        if m < 8:
                            dst = ps_rz[:, m * B : (m + 1) * B]
                        else:
                            dst = ps_n[:, (m - 8) * B : (m - 7) * B]
                        for k in range(KT):
                            nc.tensor.matmul(
                                dst,
                                lhsT=wht_sb[:, (m * KT + k) * P : (m * KT + k + 1) * P],
                                rhs=h_tile[:, k * B : (k + 1) * B],
                                start=(k == 0),
                                stop=(k == KT - 1),
                            )
                    arz = wpool.tile([P, 8 * B], FP, tag="arz")
                    nc.vector.tensor_add(_v(arz[:], 8), _v(ps_rz[:], 8), gi_rz_at(t))
                    rzt = wpool.tile([P, 8 * B], FP, tag="rz")
                    nc.scalar.activation(rzt[:, :], arz[:, :], AF.Sigmoid)
                    # b = (gh_n + b_hn) * r, per k-chunk (bias differs per chunk)
                    bt = wpool.tile([P, KT * B], FP, tag="bt")
                    for k in range(KT):
                        nc.vector.scalar_tensor_tensor(
                            bt[:, k * B : (k + 1) * B],
                            in0=ps_n[:, k * B : (k + 1) * B],
                            scalar=bhn_sb[:, k : k + 1],
                            in1=rzt[:, k * B : (k + 1) * B],
                            op0=OP.add,
                            op1=OP.mult,
                        )
                    ct = wpool.tile([P, KT * B], FP, tag="ct")
                    nc.vector.tensor_add(_v(ct[:], KT), _v(bt[:], KT), gi_n_at(t))
                    nt = wpool.tile([P, KT * B], FP, tag="nt")
                    nc.scalar.activation(nt[:, :], ct[:, :], AF.Tanh)
                    # h' = n + z*(h - n)
                    et = wpool.tile([P, KT * B], FP, tag="et")
                    nc.vector.tensor_sub(et[:, :], h_tile[:, :], nt[:, :])
                    ft = wpool.tile([P, KT * B], FP, tag="ft")
                    nc.vector.tensor_mul(ft[:, :], rzt[:, KT * B :], et[:, :])
                    nc.vector.tensor_add(h_tile[:, :], nt[:, :], ft[:, :])
                    if t == 0:
                        nc.vector.tensor_copy(acc_tile[:, :], h_tile[:, :])
                    else:
                        nc.vector.tensor_add(acc_tile[:, :], acc_tile[:, :], h_tile[:, :])

            # ---------------- Level 3: 64 nodes ----------------
            gi3 = cpool.tile([P, MT * T * NB], FP)
            # moving operand is 512 cols - exactly the fp32 cap
            compute_gi(gi3, lambda k: xt_sb[:, k * (T * NB) : (k + 1) * (T * NB)], T * NB)
            gi3v = gi3[:].rearrange("p (m t b) -> p m t b", m=MT, t=T)
            h3 = spool.tile([P, KT * NB], BF)
            acc3 = spool.tile([P, KT * NB], FP)
            gru_level(
                NB,
                h3,
                acc3,
                lambda t: gi3v[:, 0:8, t],
                lambda t: gi3v[:, 8:12, t],
                zero_h0=True,
            )

            # ---------------- Level 3 -> 2 transition ----------------
            x2 = spool.tile([P, KT * NB], BF)
            nc.scalar.mul(x2[:, :], acc3[:, :], 1.0 / A)
            h2 = spool.tile([P, KT * A], BF)
            hr2 = spool.tile([P, KT * A], FP)
            nc.vector.tensor_reduce(
                _v(hr2[:], KT),
                h3[:].rearrange("p (k j c) -> p k j c", k=KT, j=A),
                axis=mybir.AxisListType.X,
                op=OP.add,
            )
            nc.scalar.mul(h2[:, :], hr2[:, :], 1.0 / A)

            gi2 = cpool.tile([P, MT * NB], FP)
            compute_gi(gi2, lambda k: x2[:, k * NB : (k + 1) * NB], NB)
            # gi2 within-m column order is (j, t) - natural child order; the
            # step-t slice is strided.
            gi2v = gi2[:].rearrange("p (m j t) -> p m j t", m=MT, j=A)
            acc2 = spool.tile([P, KT * A], FP)
            gru_level(
                A,
                h2,
                acc2,
                lambda t: gi2v[:, 0:8, :, t],
                lambda t: gi2v[:, 8:12, :, t],
                zero_h0=False,
            )

            # ---------------- Level 2 -> 1 transition ----------------
            x1 = spool.tile([P, KT * A], BF)
            nc.scalar.mul(x1[:, :], acc2[:, :], 1.0 / A)
            h1 = spool.tile([P, KT], BF)
            hr1 = spool.tile([P, KT], FP)
            nc.vector.tensor_reduce(
                _v(hr1[:], KT),
                h2[:].rearrange("p (k j c) -> p k j c", k=KT, j=1),
                axis=mybir.AxisListType.X,
                op=OP.add,
            )
            nc.scalar.mul(h1[:, :], hr1[:, :], 1.0 / A)

            gi1 = cpool.tile([P, MT * A], FP)
            compute_gi(gi1, lambda k: x1[:, k * A : (k + 1) * A], A)
            gi1v = gi1[:].rearrange("p (m t) -> p m t", m=MT)
            acc1 = spool.tile([P, KT], FP)
            gru_level(
                1,
                h1,
                acc1,
                lambda t: gi1v[:, 0:8, t : t + 1],
                lambda t: gi1v[:, 8:12, t : t + 1],
                zero_h0=False,
            )

            out_sb = spool.tile([P, KT], FP)
            nc.scalar.mul(out_sb[:, :], acc1[:, :], 1.0 / A)
            nc.sync.dma_start(out=outp[:, :], in_=out_sb[:, :])

    nc.finalize()
    return nc


def _get_nc():
    global _BUILT
    if _BUILT is None:
        _BUILT = _build_nc()
    return _BUILT


def make_inputs(leaf_ids, embed_table, W_ih, W_hh, b_ih, b_hh):
    """Host-side shard/layout prep: gather the looked-up embedding rows and
    lay every tensor out in the on-chip transposed format."""
    leaf_ids = np.asarray(leaf_ids).astype(np.int64)
    emb = np.asarray(embed_table, dtype=np.float32)
    W_ih = np.asarray(W_ih, dtype=np.float32)
    W_hh = np.asarray(W_hh, dtype=np.float32)
    b_ih = np.asarray(b_ih, dtype=np.float32)
    b_hh = np.asarray(b_hh, dtype=np.float32)

    x = emb[leaf_ids]  # [64, 8, 512]
    # time-major batch: row b = t*64 + node
    xtm = np.ascontiguousarray(x.transpose(1, 0, 2)).reshape(T * NB, D)
    xt_in = np.ascontiguousarray(
        xtm.T.reshape(KT, P, T * NB).transpose(1, 0, 2)
    ).reshape(P, KT * T * NB)

    def pack_w(W):  # W [1536, 512] -> lhsT tiles [(m,k) major]
        WT = np.ascontiguousarray(W.T)  # [512, 1536]
        return np.ascontiguousarray(
            WT.reshape(KT, P, MT, P).transpose(1, 2, 0, 3)
        ).reshape(P, MT * KT * P)

    gbias = np.concatenate([(b_ih + b_hh)[: 2 * D], b_ih[2 * D :]])
    gb_in = np.ascontiguousarray(gbias.reshape(MT, P).T)
    bhn_in = np.ascontiguousarray(b_hh[2 * D :].reshape(KT, P).T)
    bhnb_in = np.ascontiguousarray(np.repeat(bhn_in, NB, axis=1))

    import ml_dtypes

    blob16 = np.concatenate([pack_w(W_ih), pack_w(W_hh), xt_in], axis=1).astype(
        ml_dtypes.bfloat16
    )
    blob32 = np.concatenate([gb_in, bhn_in, bhnb_in], axis=1)
    assert blob16.shape == (P, B16_COLS) and blob32.shape == (P, B32_COLS)
    return {
        "blob16": np.ascontiguousarray(blob16),
        "blob32": np.ascontiguousarray(blob32),
    }


def unpack_output(out_np):
    # out [P, KT]: element (p, k) = root dim k*128+p
    return np.ascontiguousarray(out_np.T).reshape(1, 1, D).astype(np.float32)


def kernel(leaf_ids=None, layer=None, embed_table=None, W_ih=None, W_hh=None,
           b_ih=None, b_hh=None, **_unused):
    in_map = make_inputs(leaf_ids, embed_table, W_ih, W_hh, b_ih, b_hh)
    nc = _get_nc()
    res = run_bass_kernel_spmd(nc, [in_map] * N_CORES, list(range(N_CORES)))
    return unpack_output(res.results[0]["out"])
